# revision 16
# baseline (speedup 1.0000x reference)
"""3x3 valid cross-correlation of a 4096x4096 fp32 image + scalar bias,
sharded row-wise across 8 TRN2 NeuronCores.

Strategy per core (512 output rows, 514 input rows incl. 2-row halo taken
host-side via overlapping slices -- no device collectives):
  - Row panels of 128 input rows -> 126 output rows (banded matmul):
    out[m, n] = sum_dc sum_dr w[dr, dc] * x[m+dr, n+dc]
    For each kernel column dc, a banded stationary matrix
    B_dc[k, m] = w[k-m, dc] (k-m in 0..2) gives
    (B_dc.T-free) matmul: psum[m, n] += sum_k B_dc[k, m] * x[k, n+dc].
    The 3 dc-matmuls accumulate into one PSUM bank; the column shift dc is
    folded into the moving-operand (rhs) free-dim offset.
  - x and w DRAM tensors are declared float32r so the DMA loads feed the
    PE directly (single-pass fp32 matmul, 1 cycle/row at N>=256) with no
    SBUF-to-SBUF converting copy stage.
  - The 8-row tail panel runs FIRST: its tiny load arrives immediately via
    SWDGE, its store lands mid-stream, and the kernel ends on a big
    well-spread panel store instead of a small straggler DMA. Tail tiles
    sit at partition offset 8 so SWDGE's partition swizzle routes them to
    SDMA engines {4,6} instead of piling onto engine 0 (already the
    busiest engine).
  - The 4 full-panel loads are issued up-front on the SP HWDGE ring with
    no backpressure; panel stores are [M, 2048]-column chunks, one half on
    the ACT ring and one on the SP ring (HWDGE spreads such chunks evenly
    over SDMA engines 0-13, and 14 engines at line rate still exceed the
    per-NC HBM cap).
  - Bias is fused into the PSUM->SBUF drains (ScalarE activation bias for
    one 2048-wide chunk, VectorE tensor_scalar_add for the other).
  - PE clock warmup: ~3.4us of throwaway fp32 matmuls (4 cyc/row) bridge
    the gap until the tail panel's data lands (~3us of continuous PE work
    ramps the clock to 2.4 GHz).
  - Last core overlaps core 6 by 2 rows so that all cores run an identical
    514-row program (4094 = 8*512 - 2).
"""

import numpy as np

import concourse.bacc as bacc
import concourse.mybir as mybir
from concourse import tile
from concourse.bass_utils import run_bass_kernel_spmd

H, W = 4096, 4096
KH, KW = 3, 3
OH, OW = H - KH + 1, W - KW + 1  # 4094, 4094
NCORES = 8
ROWS_PER_CORE = 512              # output rows computed per core
IN_ROWS = ROWS_PER_CORE + KH - 1  # 514 input rows per core
PANEL_OUT = 126                  # output rows per full 128-input-row panel
N_FULL_PANELS = 4                # 4 * 126 = 504
TAIL_OUT = ROWS_PER_CORE - N_FULL_PANELS * PANEL_OUT  # 8
TAIL_IN = TAIL_OUT + KH - 1      # 10
TAIL_P0 = 64                     # partition offset of the tail tiles (matmul base partition must be 0/32/64; 64 maps the tail's SWDGE DMAs to engines {1,3,5}, sparing engine 0)
COLS_PER_MM = 512                # fp32 moving-operand / PSUM-bank max
W_COLS = KW * (PANEL_OUT + TAIL_OUT)  # main bands + tail bands

_F32 = mybir.dt.float32
_F32R = mybir.dt.float32r

_PROGRAM_CACHE = None
last_results = None  # BassKernelResults of the most recent kernel() call


def _build_program():
    nc = bacc.Bacc(
        "TRN2", target_bir_lowering=False, debug=False, num_devices=NCORES
    )
    x = nc.dram_tensor("x", [IN_ROWS, W], _F32R, kind="ExternalInput")
    w = nc.dram_tensor("w", [128, W_COLS], _F32R, kind="ExternalInput")
    b = nc.dram_tensor("b", [128, 1], _F32, kind="ExternalInput")
    y = nc.dram_tensor("y", [ROWS_PER_CORE, OW], _F32, kind="ExternalOutput")

    HALF = 4 * COLS_PER_MM  # 2048

    with tile.TileContext(nc) as tc:
        with (
            tc.tile_pool(name="const", bufs=1) as cpool,
            tc.tile_pool(name="xp", bufs=5) as xpool,
            tc.tile_pool(name="op", bufs=5) as opool,
            tc.tile_pool(name="pp", bufs=2, space="PSUM") as ppool,
        ):
            # Full-panel x loads first: no dependencies, the SP ring streams
            # them back-to-back while everything else spins up.
            xts = []
            # Tail panel load (10 rows) via SWDGE at partitions 64..73.
            xtail = xpool.tile([128, W], _F32R, tag="x")
            nc.gpsimd.dma_start(
                xtail[TAIL_P0 : TAIL_P0 + TAIL_IN, :],
                x[PANEL_OUT * N_FULL_PANELS :, :],
            )

            for panel in range(N_FULL_PANELS):
                r0 = PANEL_OUT * panel
                xt = xpool.tile([128, W], _F32R, tag="x")
                nc.sync.dma_start(xt[:, :], x[r0 : r0 + 128, :])
                xts.append(xt)

            # Weights + bias ride the ACT ring (idle until the first store)
            # so they don't queue behind 8.4 MB of x loads on the SP ring.
            wtr = cpool.tile([128, W_COLS], _F32R)
            nc.scalar.dma_start(wtr[:], w[:])
            bt = cpool.tile([128, 1], _F32)
            nc.scalar.dma_start(bt[:], b[:])

            # PE clock warmup: fp32 (two-pass, 4 cyc/row) matmuls on a memset
            # tile are long-running single instructions that keep the PE busy
            # until the tail panel's data arrives (~3us of continuous PE work
            # ramps the clock to 2.4 GHz).
            wz = cpool.tile([128, COLS_PER_MM], _F32)
            nc.gpsimd.memset(wz[:], 0.0)
            pswarm = ppool.tile([128, COLS_PER_MM], _F32, tag="ps")
            for _ in range(4):
                nc.tensor.matmul(
                    pswarm[:126, :],
                    wz[:, :126],
                    wz[:, :],
                    start=True,
                    stop=True,
                )

            for panel in range(N_FULL_PANELS):
                r0 = PANEL_OUT * panel
                xt = xts[panel]
                ot = opool.tile([128, OW], _F32, tag="ot")
                for c in range(2):
                    # One 4-bank PSUM tile per 2048-col half: each of the 4
                    # matmul groups lands in its own bank, then a single wide
                    # drain covers the half.
                    ps = ppool.tile([128, HALF], _F32, tag="ps")
                    s0 = c * HALF
                    sw = min(HALF, OW - s0)  # 2048 / 2046
                    for jj in range(4):
                        c0 = s0 + jj * COLS_PER_MM
                        N = min(COLS_PER_MM, OW - c0)
                        lc0 = jj * COLS_PER_MM
                        for dc in range(KW):
                            nc.tensor.matmul(
                                ps[:PANEL_OUT, lc0 : lc0 + N],
                                wtr[:128, dc * PANEL_OUT : dc * PANEL_OUT + PANEL_OUT],
                                xt[:128, c0 + dc : c0 + dc + N],
                                start=(dc == 0),
                                stop=(dc == KW - 1),
                            )
                    # Drain PSUM on alternating engines so neither ScalarE
                    # nor VectorE becomes the bottleneck.
                    if c % 2 == 0:
                        nc.scalar.activation(
                            ot[:PANEL_OUT, s0 : s0 + sw],
                            ps[:PANEL_OUT, :sw],
                            mybir.ActivationFunctionType.Identity,
                            bias=bt[:PANEL_OUT, :],
                        )
                    else:
                        nc.vector.tensor_scalar_add(
                            ot[:PANEL_OUT, s0 : s0 + sw],
                            ps[:PANEL_OUT, :sw],
                            bt[:PANEL_OUT, :],
                        )
                # Panel stores as [126, 2048]-column chunks: HWDGE spreads
                # these evenly over SDMA engines 0-13. Half rides ACT, half
                # rides SP (queued FIFO behind the loads -- each store is
                # drain-ready before the ring reaches it), so two descriptor
                # streams keep the engines fed.
                nc.scalar.dma_start(
                    y[r0 : r0 + PANEL_OUT, :HALF], ot[:PANEL_OUT, :HALF]
                )
                nc.sync.dma_start(
                    y[r0 : r0 + PANEL_OUT, HALF:OW], ot[:PANEL_OUT, HALF:OW]
                )
            # Tail panel compute last: 8 output rows; sources read partitions
            # 64..73 (SWDGE-loaded, engines {1,3,5}), PSUM dst at 0..7.
            r0 = PANEL_OUT * N_FULL_PANELS  # 504
            ot_t = opool.tile([128, OW], _F32, tag="ot")
            for c in range(2):
                ps = ppool.tile([128, HALF], _F32, tag="ps")
                s0 = c * HALF
                sw = min(HALF, OW - s0)
                for jj in range(4):
                    c0 = s0 + jj * COLS_PER_MM
                    N = min(COLS_PER_MM, OW - c0)
                    lc0 = jj * COLS_PER_MM
                    for dc in range(KW):
                        nc.tensor.matmul(
                            ps[:TAIL_OUT, lc0 : lc0 + N],
                            wtr[
                                TAIL_P0 : TAIL_P0 + TAIL_IN,
                                KW * PANEL_OUT + dc * TAIL_OUT : KW * PANEL_OUT
                                + dc * TAIL_OUT
                                + TAIL_OUT,
                            ],
                            xtail[TAIL_P0 : TAIL_P0 + TAIL_IN, c0 + dc : c0 + dc + N],
                            start=(dc == 0),
                            stop=(dc == KW - 1),
                        )
                if c % 2 == 0:
                    nc.scalar.activation(
                        ot_t[:TAIL_OUT, s0 : s0 + sw],
                        ps[:TAIL_OUT, :sw],
                        mybir.ActivationFunctionType.Identity,
                        bias=bt[:TAIL_OUT, :],
                    )
                else:
                    nc.vector.tensor_scalar_add(
                        ot_t[:TAIL_OUT, s0 : s0 + sw],
                        ps[:TAIL_OUT, :sw],
                        bt[:TAIL_OUT, :],
                    )
            # Tail stores ride HWDGE (lowest completion latency for the
            # final DMA), split across both rings.
            nc.scalar.dma_start(y[r0:, :HALF], ot_t[:TAIL_OUT, :HALF])
            nc.sync.dma_start(y[r0:, HALF:OW], ot_t[:TAIL_OUT, HALF:OW])

    nc.compile()
    return nc


def _banded_weights(weight: np.ndarray) -> np.ndarray:
    """lhsT bands laid out as [128, KW*PANEL_OUT + KW*TAIL_OUT].

    Main bands (full panels, contraction partitions 0..127):
      wT[k, dc*PANEL_OUT + m] = weight[k - m, dc] for 0 <= k - m < KH.
    Tail bands (contraction partitions 64..73, output partitions 64..71):
      wT[64 + k, KW*PANEL_OUT + dc*TAIL_OUT + m] = weight[k - m, dc].
    """
    wT = np.zeros((128, W_COLS), np.float32)
    m = np.arange(PANEL_OUT)
    for dc in range(KW):
        for d in range(KH):
            wT[m + d, dc * PANEL_OUT + m] = weight[d, dc]
    mt = np.arange(TAIL_OUT)
    for dc in range(KW):
        for d in range(KH):
            wT[TAIL_P0 + mt + d, KW * PANEL_OUT + dc * TAIL_OUT + mt] = weight[d, dc]
    return wT


def _install_ntff_hook():
    """Shim antenv.axon_hooks so run_bass_kernel_spmd(trace=True) can find
    the axon NTFF profiling hook (the image's antenv lacks axon_hooks)."""
    import sys
    import types

    try:
        from antenv.axon_hooks import get_axon_ntff_profile_hook  # noqa: F401

        return
    except ImportError:
        pass
    import antenv
    from trn_agent_boot.trn_boot import _ntff_profile_via_ctypes

    hook = _ntff_profile_via_ctypes("/opt/axon/libaxon_pjrt.so")
    mod = types.ModuleType("antenv.axon_hooks")
    mod._hook = hook
    mod.set_axon_ntff_profile_hook = lambda h: setattr(mod, "_hook", h)
    mod.get_axon_ntff_profile_hook = lambda: mod._hook
    sys.modules["antenv.axon_hooks"] = mod
    antenv.axon_hooks = mod


def kernel(x, weight, bias, _trace=False, _trace_cores=None):
    global _PROGRAM_CACHE, last_results
    if _trace:
        _install_ntff_hook()
    x = np.ascontiguousarray(np.asarray(x, dtype=np.float32))
    weight = np.asarray(weight, dtype=np.float32)
    bias = np.asarray(bias, dtype=np.float32)

    if _PROGRAM_CACHE is None:
        _PROGRAM_CACHE = _build_program()
    nc = _PROGRAM_CACHE

    wT = _banded_weights(weight)
    bb = np.full((128, 1), bias[0], np.float32)

    in_maps = []
    for i in range(NCORES):
        r0 = i * ROWS_PER_CORE if i < NCORES - 1 else H - IN_ROWS
        in_maps.append(
            {"x": np.ascontiguousarray(x[r0 : r0 + IN_ROWS]), "w": wT, "b": bb}
        )

    kwargs = {}
    if _trace:
        kwargs["trace"] = True
        kwargs["trace_cores"] = (
            list(range(NCORES)) if _trace_cores is None else _trace_cores
        )
    res = run_bass_kernel_spmd(nc, in_maps, core_ids=list(range(NCORES)), **kwargs)
    last_results = res

    out = np.empty((OH, OW), np.float32)
    for i in range(NCORES - 1):
        out[i * ROWS_PER_CORE : (i + 1) * ROWS_PER_CORE] = res.results[i]["y"]
    tail_rows = OH - (NCORES - 1) * ROWS_PER_CORE  # 510
    out[(NCORES - 1) * ROWS_PER_CORE :] = res.results[-1]["y"][
        ROWS_PER_CORE - tail_rows :
    ]
    return out
